# revision 35
# baseline (speedup 1.0000x reference)
"""Trainium2 Bass kernel for nn_EnhancedLocalAttention.

Reference semantics (B=4, L=4096, C=1024, H=16, D=64, WIN=256, step=128):
  qkv = x @ W_qkv + b_qkv -> q,k,v [B,H,L,D]
  overlapping windows n: tokens [n*128, n*128+256)
  per (b,h,n): S = (Q_win^T K_win)/8  (D x D, contracted over the 256 window
  tokens), P = softmax(S, axis=-1), O = P @ V_win^T  (D x W)
  regroup: rows of reshape(O, [256, 64]) laid at tokens n*256..n*256+255,
  slice to L -> only windows 0..15 survive; then @ W_out + b_out.

Sharding: 8 cores = (4 batches) x (2 window-halves of 8 windows each).
Each core consumes 9 x 128-token chunks and produces 2048 output rows.

Schedule (v2): weights stream as 16 column-slice-half DMAs ordered q,k,v,
W_out; the QKV projection runs slice-major across all 9 chunks so the PE
consumes each weight slice the moment it lands (no weight stall, no head-of-
line blocking).  After the q/k passes, rounds interleave the v projection +
V^T transposes with window attention and the previous window's out-
projection, so the in-order PE stream always has dense matmul work covering
the softmax chain latency.  Softmax is diagonal-packed: all 8 head-pairs of
a window share one PSUM bank, giving a single exp / segmented-rowsum /
reciprocal for the whole window.  PSUM->SBUF copies are spread across
DVE/ACT/Pool to keep any one queue short.
"""

import threading

import numpy as np

import concourse.bacc as bacc
import concourse.masks as masks
import concourse.mybir as mybir
import concourse.tile as tile
from concourse._compat import get_trn_type
from concourse.bass_utils import run_bass_kernel_spmd

F32 = mybir.dt.float32
F16 = mybir.dt.float16
EXP = mybir.ActivationFunctionType.Exp
AX_X = mybir.AxisListType.X
ALU_ADD = mybir.AluOpType.add

B, L, C = 4, 4096, 1024
H, D, WIN, STEP = 16, 64, 256, 128
NCHUNK = 9            # 128-token chunks per core
NWIN = 8              # windows per core
TOK = NCHUNK * 128    # 1152 input tokens per core
OUT_ROWS = NWIN * 256 # 2048 output rows per core


def interleave(a, b):
    """Merge two unit lists proportionally (Bresenham)."""
    if not b:
        return list(a)
    if not a:
        return list(b)
    out = []
    ia = ib = 0
    while ia < len(a) or ib < len(b):
        if ib >= len(b) or (ia < len(a) and ia * len(b) <= ib * len(a)):
            out.append(a[ia]); ia += 1
        else:
            out.append(b[ib]); ib += 1
    return out


def build_program(with_bias=True):
    nc = bacc.Bacc(
        get_trn_type() or "TRN2",
        target_bir_lowering=False,
        debug=False,
        num_devices=8,
    )
    xs = nc.dram_tensor("xs", [TOK, C], F32, kind="ExternalInput")
    wqkv = nc.dram_tensor("wqkv", [C, 3 * C], F32, kind="ExternalInput")
    bqkv = nc.dram_tensor("bqkv", [3 * C], F32, kind="ExternalInput")
    wout = nc.dram_tensor("wout", [C, C], F32, kind="ExternalInput")
    bout = nc.dram_tensor("bout", [C], F32, kind="ExternalInput")
    out = nc.dram_tensor("out", [OUT_ROWS, C], F32, kind="ExternalOutput")

    from contextlib import ExitStack

    with tile.TileContext(nc) as tc, ExitStack() as ctx:
        pool = lambda name, bufs, **kw: ctx.enter_context(
            tc.tile_pool(name=name, bufs=bufs, **kw)
        )
        const_pool = pool("const", 1)
        ws_pool = pool("ws", 24)     # qkv weight slice-quarters [128,1024]
        wo_pool = pool("wo", 4)      # wout slice-halves [128,2048]
        wos_pool = pool("wos", 1)    # wout f32 staging
        x_pool = pool("x", 3)        # x chunks f32 (sync-queue DMA)
        x16_pool = pool("x16", 3)    # x chunks cast to f16
        xt_pool = pool("xt", 9)      # x^T per chunk [128,1024]
        q_pool = pool("q", 9)
        k_pool = pool("k", 9)
        v_pool = pool("v", 3)
        vt_pool = pool("vt", 4)      # V^T per chunk [128,1024]
        pe_pool = pool("pe", 2)      # p_exp_all [128,512]
        at_pool = pool("at", 18)     # p_n / ptsb [128,64] f16 (2 windows live)
        st_pool = pool("st", 4)      # ssum/rs [128,8] f32
        yt_pool = pool("yt", 18)
        o_pool = pool("o", 4)        # out staging halves [128,512] f32
        # PSUM: bank-granular pools, 8 banks total.
        ps_acc = ctx.enter_context(tc.tile_pool(name="ps_acc", bufs=4, space="PSUM"))
        ps_s = ctx.enter_context(tc.tile_pool(name="ps_s", bufs=2, space="PSUM"))
        ps_tr = ctx.enter_context(tc.tile_pool(name="ps_tr", bufs=1, space="PSUM"))
        ps_pt = ctx.enter_context(tc.tile_pool(name="ps_pt", bufs=1, space="PSUM"))

        # --- constants (identity is emitted later, after the first DMA
        # issues, so the gpsimd queue starts streaming weights immediately)
        idf16 = const_pool.tile([128, 128], F16, tag="idf16", name="idf16")
        if with_bias:
            ones = const_pool.tile([1, 128], F16, tag="ones", name="ones")
            nc.vector.memset(ones[:], 1.0)
            bq_sb = const_pool.tile([1, 3 * C], F16, tag="bq", name="bq_sb")
            nc.gpsimd.dma_start(bq_sb[:], bqkv.ap().rearrange("(a f) -> a f", a=1))
            bo_sb = const_pool.tile([1, C], F16, tag="bo", name="bo_sb")
            nc.gpsimd.dma_start(bo_sb[:], bout.ap().rearrange("(a f) -> a f", a=1))

        # --- DMA issue units (all casting DMAs must come from gpsimd) ---
        x_pre = [None] * NCHUNK

        def u_x_dma(r):
            def f():
                x_t = x_pool.tile([128, C], F32, tag="x", name=f"x{r}")
                nc.sync.dma_start(x_t[:], xs.ap()[r * 128 : (r + 1) * 128, :])
                x_pre[r] = x_t
            return f

        # slice j = cols [j*512,(j+1)*512); half h = rows [h*512,(h+1)*512)
        # laid as 4 row-blocks of 128 side by side:
        # tile[p, cb*512 + f] = W[h*512 + cb*128 + p, j*512 + f]
        wq_t = [[None] * 4 for _ in range(6)]     # [slice][quarter]
        wo_t = [[None, None] for _ in range(2)]

        def u_w_dma(j, q):
            """Quarter-granular (0.5 MB) so the first consumer starts early."""
            def f():
                t = ws_pool.tile([128, 1024], F16, tag="ws", name=f"wq{j}_{q}")
                wq_t[j][q] = t
                src = wqkv.ap()[q * 256 : (q + 1) * 256, j * 512 : (j + 1) * 512]
                nc.gpsimd.dma_start(
                    t[:].rearrange("p (a f) -> p a f", a=2),
                    src.rearrange("(a p) f -> p a f", p=128),
                )
            return f

        # W_out rides the scalar queue as f32 (no cast in DMA), then ACT
        # casts it into the f16 slice tile.  Keeps gpsimd free for q,k,v.
        wos_cur = {}

        def u_wo_dma(j, h):
            def f():
                stg = wos_pool.tile([128, 2048], F32, tag="wos", name="wos")
                wos_cur[(j, h)] = stg
                src = wout.ap()[h * 512 : (h + 1) * 512, j * 512 : (j + 1) * 512]
                nc.scalar.dma_start(
                    stg[:].rearrange("p (a f) -> p a f", a=4),
                    src.rearrange("(a p) f -> p a f", p=128),
                )
            return f

        def u_wo_cast(j, h):
            def f():
                t = wo_pool.tile([128, 2048], F16, tag="wo", name=f"wo{j}_{h}")
                wo_t[j][h] = t
                nc.scalar.copy(t[:], wos_cur.pop((j, h))[:])
            return f

        xt_all = [None] * NCHUNK
        q_sb = [None] * NCHUNK
        k_sb = [None] * NCHUNK
        v_sb = [None] * NCHUNK
        vt_sb = [None] * NCHUNK

        def u_xt(r):
            def f():
                x16 = x16_pool.tile([128, C], F16, tag="x16", name="x16")
                nc.vector.tensor_copy(x16[:], x_pre[r][:])
                tp = ps_tr.tile([128, C], F16, tag="tr", name="tp")
                for cb in range(8):
                    nc.tensor.transpose(
                        tp[:, cb * 128 : (cb + 1) * 128],
                        x16[:, cb * 128 : (cb + 1) * 128],
                        idf16[:],
                    )
                xtt = xt_pool.tile([128, C], F16, tag="xt", name=f"xt{r}")
                nc.scalar.copy(xtt[:], tp[:])
                xt_all[r] = xtt
            return f

        # slice-major QKV projection pass units.  slice j: 0,1=q  2,3=k  4,5=v
        pq_cur = {}

        def u_mmA(j, r):
            def f():
                pq = ps_acc.tile([128, 512], F32, tag="acc", name="pq")
                pq_cur[(j, r)] = pq
                for cb in range(4):
                    nc.tensor.matmul(
                        pq[:],
                        xt_all[r][:, cb * 128 : (cb + 1) * 128],
                        wq_t[j][cb // 2][:, (cb % 2) * 512 : (cb % 2) * 512 + 512],
                        start=(cb == 0),
                        stop=False,
                    )
            return f

        def u_mmB(j, r, cast_engine):
            def f():
                pq = pq_cur.pop((j, r))
                for cb in range(4):
                    nc.tensor.matmul(
                        pq[:],
                        xt_all[r][:, (cb + 4) * 128 : (cb + 5) * 128],
                        wq_t[j][2 + cb // 2][:, (cb % 2) * 512 : (cb % 2) * 512 + 512],
                        start=False,
                        stop=(not with_bias and cb == 3),
                    )
                if with_bias:
                    nc.tensor.matmul(
                        pq[:],
                        ones[:, :],
                        bq_sb[:, j * 512 : (j + 1) * 512],
                        start=False,
                        stop=True,
                    )
                kind, half = divmod(j, 2)
                store = (q_sb, k_sb, v_sb)[kind]
                if store[r] is None:
                    p = (q_pool, k_pool, v_pool)[kind]
                    store[r] = p.tile(
                        [128, C], F16, tag="qkv"[kind], name=f"{'qkv'[kind]}{r}"
                    )
                dst = store[r][:, half * 512 : (half + 1) * 512]
                if cast_engine == "act":
                    nc.scalar.copy(dst, pq[:])
                else:
                    nc.vector.tensor_copy(dst, pq[:])
            return f

        # V^T computed directly: vt[vd, t] = sum_c W_v[c, vd] * xT[c, t].
        # No v projection, no PE transposes, no intermediate v tile.
        vt_ps = {}

        def u_vtd(r, bank):
            def f():
                vps = ps_s.tile([128, 512], F32, tag="s", name="vps")
                vt_ps[(r, bank)] = vps
                for vbl in range(4):
                    vb = bank * 4 + vbl
                    j = 4 + vb // 4
                    fo = (vb % 4) * 128
                    for cb in range(8):
                        nc.tensor.matmul(
                            vps[:, vbl * 128 : (vbl + 1) * 128],
                            wq_t[j][cb // 2][
                                :, (cb % 2) * 512 + fo : (cb % 2) * 512 + fo + 128
                            ],
                            xt_all[r][:, cb * 128 : (cb + 1) * 128],
                            start=(cb == 0),
                            stop=(not with_bias and cb == 7),
                        )
                    if with_bias:
                        nc.tensor.matmul(
                            vps[:, vbl * 128 : (vbl + 1) * 128],
                            bq_sb[:, 2048 + vb * 128 : 2048 + (vb + 1) * 128],
                            ones[:, :],
                            start=False,
                            stop=True,
                        )
                if bank == 0:
                    vt_sb[r] = vt_pool.tile([128, C], F16, tag="vt", name=f"vt{r}")
                nc.scalar.copy(
                    vt_sb[r][:, bank * 512 : (bank + 1) * 512], vps[:]
                )
                vt_ps.pop((r, bank))
            return f

        # --- window attention units (3-stage pipeline) ---
        # Round r:  S+softmax of window r-1  |  P^T + O of window r-2  |
        #           out-projection of window r-3  |  v projection chunk r.
        # The softmax chain of a window has an entire round (~12us) to drain
        # before its P^T is needed, so the in-order PE queue never stalls.
        win_st = [dict() for _ in range(NWIN)]
        yt_wins = [[None] * 8 for _ in range(NWIN)]

        # S layout: head-pair hp lives at partitions [0:64) (hp<4) or [64:128)
        # (hp>=4), columns (hp%4)*128 .. +128 holding [h0 e | h1 e].  Rows = d.
        # This makes P^T a SINGLE [64,128]->[128,64] transpose per head pair,
        # landing exactly in the (e-on-partitions, per-head-halves) layout the
        # O matmul wants.
        def u_s(n, hp):
            def f():
                st = win_st[n]
                if hp == 0:
                    st["s"] = ps_s.tile([128, 512], F32, tag="s", name="s_all")
                s = st["s"]
                po = 0 if hp < 4 else 64
                cb = (hp % 4) * 128
                for hl in range(2):
                    h = 2 * hp + hl
                    for rr, b0, b1 in ((n, True, False), (n + 1, False, True)):
                        nc.tensor.matmul(
                            s[po : po + 64, cb + hl * 64 : cb + hl * 64 + 64],
                            q_sb[rr][:, h * 64 : (h + 1) * 64],
                            k_sb[rr][:, h * 64 : (h + 1) * 64],
                            start=b0,
                            stop=b1,
                        )
            return f

        def u_soft(n):
            def f():
                st = win_st[n]
                s = st.pop("s")
                p_exp = pe_pool.tile([128, 512], F16, tag="pe", name="p_exp")
                nc.scalar.activation(p_exp[:], s[:], EXP, scale=0.125)
                ssum = st_pool.tile([128, 8], F32, tag="ssum", name="ssum")
                nc.vector.tensor_reduce(
                    ssum[:].rearrange("p (s o) -> p s o", o=1),
                    p_exp[:].rearrange("p (s e) -> p s e", s=8),
                    AX_X,
                    ALU_ADD,
                )
                rs = st_pool.tile([128, 8], F32, tag="rs", name="rs")
                nc.vector.reciprocal(rs[:], ssum[:])
                st["p_n"] = []
                for hp in range(8):
                    po = 0 if hp < 4 else 64
                    cb = (hp % 4) * 128
                    p_n = at_pool.tile([128, 128], F16, tag="p_n", name="p_n")
                    for hl in range(2):
                        nc.vector.tensor_scalar_mul(
                            p_n[po : po + 64, hl * 64 : hl * 64 + 64],
                            p_exp[po : po + 64, cb + hl * 64 : cb + hl * 64 + 64],
                            rs[po : po + 64, (hp % 4) * 2 + hl : (hp % 4) * 2 + hl + 1],
                        )
                    st["p_n"].append(p_n)
            return f

        def u_pt(n, hp):
            def f():
                st = win_st[n]
                if hp == 0:
                    st["ptp"] = ps_pt.tile([128, 512], F16, tag="pt", name="ptp")
                ptp = st["ptp"]
                po = 0 if hp < 4 else 64
                p_n = st["p_n"][hp]
                nc.tensor.transpose(
                    ptp[:, hp * 64 : (hp + 1) * 64],
                    p_n[po : po + 64, :],
                    idf16[po : po + 64, po : po + 64],
                )
                ptsb = at_pool.tile([128, 64], F16, tag="ptsb", name="ptsb")
                nc.vector.tensor_copy(ptsb[:], ptp[:, hp * 64 : (hp + 1) * 64])
                st[("ptsb", hp)] = ptsb
            return f

        def u_o(n, hp):
            def f():
                st = win_st[n]
                h0 = 2 * hp
                ptsb = st.pop(("ptsb", hp))
                if hp % 2 == 0:
                    st["yb"] = ps_acc.tile([128, 512], F32, tag="acc", name="yb")
                yb = st["yb"]
                off = (hp % 2) * 256
                for h, po in ((h0, 0), (h0 + 1, 64)):
                    rh = ptsb[po : po + 64, :]
                    for wq in range(4):
                        vtt = vt_sb[n + wq // 2]
                        c0 = (h // 2) * 128 + (wq % 2) * 64
                        nc.tensor.matmul(
                            yb[po : po + 64, off + wq * 64 : off + (wq + 1) * 64],
                            vtt[po : po + 64, c0 : c0 + 64],
                            rh,
                            start=True,
                            stop=True,
                        )
                ytt = yt_pool.tile([128, 256], F16, tag="yt", name="ytt")
                # Y^T[c, d*4+wq] = ypsum[c, wq*64+d]  (torch-unfold regroup)
                nc.scalar.copy(
                    ytt[:].rearrange("p (b a) -> p a b", a=4),
                    yb[:, off : off + 256].rearrange("p (a b) -> p a b", a=4),
                )
                yt_wins[n][hp] = ytt
            return f

        def u_op(n, th, half):
            def f():
                st = win_st[n]
                yt_prev = yt_wins[n]
                if half == 0:
                    st[("po", th)] = [
                        ps_acc.tile([128, 512], F32, tag="acc", name=f"pom{i}")
                        for i in range(2)
                    ]
                po_m = st[("po", th)]
                for cb in range(4):
                    for mi in range(2):
                        nc.tensor.matmul(
                            po_m[mi][:],
                            yt_prev[cb + 4 * half][:, th * 128 : (th + 1) * 128],
                            wo_t[mi][half][:, cb * 512 : (cb + 1) * 512],
                            start=(half == 0 and cb == 0),
                            stop=(half == 1 and not with_bias and cb == 3),
                        )
                if half == 1:
                    if with_bias:
                        for mi in range(2):
                            nc.tensor.matmul(
                                po_m[mi][:],
                                ones[:, :],
                                bo_sb[:, mi * 512 : (mi + 1) * 512],
                                start=False,
                                stop=True,
                            )
                    st.pop(("po", th))
                    row = n * 256 + th * 128
                    for mi in range(2):
                        ot = o_pool.tile([128, 512], F32, tag="o", name="ot")
                        nc.vector.tensor_copy(ot[:], po_m[mi][:])
                        nc.sync.dma_start(
                            out.ap()[row : row + 128, mi * 512 : (mi + 1) * 512],
                            ot[:],
                        )
            return f

        # =========== emission ===========
        # Phase QK: DMA issues (gpsimd) interleaved with x transposes and the
        # slice-major q/k passes so every engine queue stays in arrival order.
        def u_id():
            def f():
                masks.make_identity(nc, idf16[:])
            return f

        emit = [
            u_w_dma(0, 0), u_w_dma(0, 1), u_w_dma(0, 2), u_w_dma(0, 3),
            u_x_dma(0), u_x_dma(1),
            u_id(),
            u_x_dma(2), u_wo_dma(0, 0),
            u_xt(0), u_w_dma(1, 0),
            u_xt(1), u_w_dma(1, 1),
            u_xt(2), u_x_dma(3),
            u_mmA(0, 0), u_w_dma(1, 2), u_mmB(0, 0, "dve"),
            u_xt(3), u_x_dma(4),
            u_mmA(0, 1), u_w_dma(1, 3), u_mmB(0, 1, "dve"),
            u_xt(4), u_x_dma(5),
            u_mmA(0, 2), u_w_dma(2, 0), u_mmB(0, 2, "dve"),
            u_xt(5), u_x_dma(6),
            u_mmA(0, 3), u_w_dma(2, 1), u_mmB(0, 3, "dve"),
            u_xt(6), u_x_dma(7),
            u_mmA(0, 4), u_w_dma(2, 2), u_mmB(0, 4, "dve"),
            u_xt(7), u_x_dma(8),
            u_mmA(0, 5), u_w_dma(2, 3), u_mmB(0, 5, "dve"),
            u_xt(8),
            u_mmA(0, 6), u_w_dma(3, 0), u_mmB(0, 6, "dve"),
            u_mmA(0, 7), u_w_dma(3, 1), u_mmB(0, 7, "dve"),
            u_mmA(0, 8), u_w_dma(3, 2), u_mmB(0, 8, "dve"),
            u_w_dma(3, 3),
        ]
        wv_seq = [u_w_dma(4, q) for q in range(4)] + [u_w_dma(5, q) for q in range(4)]
        # wo DMA/cast pairs ride the scalar queue; casts are emitted late so
        # they never block ACT behind an in-flight DMA.
        wo_seq = [
            u_wo_cast(0, 0), u_wo_dma(0, 1),
            u_wo_cast(0, 1), u_wo_dma(1, 0),
            u_wo_cast(1, 0), u_wo_dma(1, 1),
            u_wo_cast(1, 1),
        ]
        for j in range(1, 4):
            for r in range(NCHUNK):
                emit.append(u_mmA(j, r))
                emit.append(u_mmB(j, r, "dve"))
                if r % 2 == 0 and wv_seq:
                    emit.append(wv_seq.pop(0))
                if (r == 4 or r == 8) and wo_seq:
                    emit.append(wo_seq.pop(0))
                    if wo_seq:
                        emit.append(wo_seq.pop(0))
        for u in emit:
            u()

        # Phase rounds: 3-stage window pipeline + v projection.
        for r in range(NCHUNK + 2):
            head = []   # S+softmax of window r-1, P^T of window r-2
            body = []   # O of window r-2
            fill = []   # out-proj of window r-3, v projection of chunk r
            if 0 <= r - 1 < NWIN:
                head += [u_s(r - 1, hp) for hp in range(8)]
                head.append(u_soft(r - 1))
            if 0 <= r - 2 < NWIN:
                head += [u_pt(r - 2, hp) for hp in range(8)]
                body += [u_o(r - 2, hp) for hp in range(8)]
            if 0 <= r - 3 < NWIN:
                fill += [
                    u_op(r - 3, 0, 0), u_op(r - 3, 0, 1),
                    u_op(r - 3, 1, 0), u_op(r - 3, 1, 1),
                ]
            if r < NCHUNK:
                fill += [u_vtd(r, 0), u_vtd(r, 1)]
            for u in head + interleave(body, fill):
                u()

    nc.compile()
    return nc


_CACHE = {}
_LOCK = threading.Lock()


def _get_program(with_bias=True):
    key = f"nc_bias{with_bias}"
    with _LOCK:
        if key not in _CACHE:
            _CACHE[key] = build_program(with_bias=with_bias)
        return _CACHE[key]


def kernel(x, W_qkv, b_qkv, W_out, b_out):
    x = np.asarray(x, dtype=np.float32)
    W_qkv = np.asarray(W_qkv, dtype=np.float32)
    b_qkv = np.asarray(b_qkv, dtype=np.float32)
    W_out = np.asarray(W_out, dtype=np.float32)
    b_out = np.asarray(b_out, dtype=np.float32)

    with_bias = bool(np.any(b_qkv)) or bool(np.any(b_out))
    nc = _get_program(with_bias=with_bias)
    in_maps = []
    for cid in range(8):
        b, half = cid // 2, cid % 2
        t0 = half * NWIN * STEP
        in_maps.append(
            {
                "xs": np.ascontiguousarray(x[b, t0 : t0 + TOK, :]),
                "wqkv": W_qkv,
                "bqkv": b_qkv,
                "wout": W_out,
                "bout": b_out,
            }
        )
    res = run_bass_kernel_spmd(nc, in_maps, core_ids=list(range(8)))
    out_full = np.empty((B, L, C), dtype=np.float32)
    for cid in range(8):
        b, half = cid // 2, cid % 2
        out_full[b, half * OUT_ROWS : (half + 1) * OUT_ROWS, :] = res.results[cid][
            "out"
        ]
    return out_full
